# revision 7
# baseline (speedup 1.0000x reference)
"""Trainium2 Bass kernel for CustomBCEWithLogitsLoss (topk masking).

Math: with e = softplus(l) - l*t (elementwise BCE-with-logits),
  out = mean_all(e) + BCE_L * mean_{top20-by-logit per row}(e)
since top-k of sigmoid(logits) = top-k of logits, and the reference's
top-k BCE term equals e at those positions (-100 clamps never bind for
|l| < 100).

Per core (8-way batch shard, 512 rows = 4 tiles of [128, 10000]):
  DMA(SP): L in 2 half DMAs, T in 4 column-chunk DMAs
  ACT:    EB_c = Ln(Exp(L_c)+1) = softplus (accum -> sum sp),
          Copy(LT_c) (accum -> sum l*t); all three funcs forced into the
          natural_log_exp_and_others table (one table load, no thrash)
  GPSIMD: LT_c = L_c * T_c; tail slice of EB_c -= LT_c
  DVE:    head slice of EB_c -= LT_c (-> e); 16x max8 over 625-col chunks
          -> 128 candidates/row; 3x(max8+match_replace) cascade -> tau =
          20th largest logit; ME_c = (L_c >= tau) * e_c with accum
Exactness: per-chunk 8th-largest (ch8) and the 21st candidate (tau2) are
output; host flags rows where max(ch8) >= tau (candidate set may have
missed a top-20 value) or tau2 == tau (boundary tie) and recomputes them
exactly (expected ~1 row in 1e5). Host combines partials in f64.
"""

import numpy as np

B, N, K = 4096, 10000, 20
NCORES = 8
R = B // NCORES          # rows per core
P = 128                  # partitions
NT = R // P              # tiles per core
CS = 2500                # streaming chunk width (4 chunks per row)
NCH = N // CS
CCH = 16                 # candidate chunks per row
W = N // CCH             # candidate chunk width (625)
SUB_DVE = 1500           # columns of each e-subtract chunk done on DVE
SLOTS = 32               # per-tile output slots
NEG_INF = -1.0e30
ACT_TABLE = "natural_log_exp_and_others"

_PROGRAM = None


def _build_program():
    import concourse.bacc as bacc
    import concourse.tile as tile
    import concourse.mybir as mybir
    from concourse.hw_specs import get_activation_tables

    nc = bacc.Bacc("TRN2", target_bir_lowering=False, debug=False)
    f32 = mybir.dt.float32
    logits = nc.dram_tensor("logits", [R, N], f32, kind="ExternalInput")
    targets = nc.dram_tensor("targets", [R, N], f32, kind="ExternalInput")
    out = nc.dram_tensor("partials", [P, NT * SLOTS], f32,
                         kind="ExternalOutput")
    Lr = logits.ap().rearrange("(t p) n -> t p n", p=P)
    Tr = targets.ap().rearrange("(t p) n -> t p n", p=P)

    AF = mybir.ActivationFunctionType
    OP = mybir.AluOpType

    with tile.TileContext(nc) as tc:
        with (
            tc.tile_pool(name="pL", bufs=2) as pL,
            tc.tile_pool(name="pT", bufs=2) as pT,
            tc.tile_pool(name="pE", bufs=1) as pE,
            tc.tile_pool(name="pLT", bufs=2) as pLT,
            tc.tile_pool(name="small", bufs=2) as small,
            tc.tile_pool(name="outp", bufs=1) as outp,
        ):
            OUT = outp.tile([P, NT * SLOTS], f32)
            nc.gpsimd.memset(OUT, 0.0)
            for t in range(NT):
                s0 = t * SLOTS
                Lt = pL.tile([P, N], f32, tag="L")
                nc.sync.dma_start(Lt[:, :N // 2], Lr[t][:, :N // 2])
                nc.sync.dma_start(Lt[:, N // 2:], Lr[t][:, N // 2:])
                EB = pE.tile([P, N], f32, tag="EB")
                for c in range(NCH):
                    cl = c * CS
                    Lc = Lt[:, cl:cl + CS]
                    Ec = EB[:, cl:cl + CS]
                    Tc = pT.tile([P, CS], f32, tag="T")
                    nc.sync.dma_start(Tc, Tr[t][:, cl:cl + CS])
                    LTc = pLT.tile([P, CS], f32, tag="LT")
                    nc.gpsimd.tensor_mul(LTc, Lc, Tc)
                    # softplus into EB: Exp, then Ln(x+1) in place
                    nc.scalar.activation(Ec, Lc, AF.Exp)
                    nc.scalar.activation(Ec, Ec, AF.Ln, bias=1.0, scale=1.0,
                                         accum_out=OUT[:, s0 + c:s0 + c + 1])
                    # row sum of l*t via ACT copy-accum (out over dead Tc)
                    nc.scalar.activation(Tc, LTc, AF.Copy,
                                         accum_out=OUT[:, s0 + 4 + c:s0 + 5 + c])
                    # e = softplus - l*t, split DVE/GPSIMD
                    nc.vector.tensor_sub(Ec[:, :SUB_DVE], Ec[:, :SUB_DVE],
                                         LTc[:, :SUB_DVE])
                    nc.gpsimd.tensor_sub(Ec[:, SUB_DVE:], Ec[:, SUB_DVE:],
                                         LTc[:, SUB_DVE:])

                # top-20 threshold: per-chunk top-8, then cascade on cand
                cand = small.tile([P, CCH * 8], f32, tag="cand")
                for c in range(CCH):
                    nc.vector.max(out=cand[:, c * 8:(c + 1) * 8],
                                  in_=Lt[:, c * W:(c + 1) * W])
                # 8th-largest of each chunk -> exactness check channel
                cv = cand[:].rearrange("p (c k) -> p c k", k=8)
                nc.gpsimd.tensor_copy(out=OUT[:, s0 + 16:s0 + 32],
                                      in_=cv[:, :, 7:8])
                m1 = small.tile([P, 8], f32, tag="m1")
                m2 = small.tile([P, 8], f32, tag="m2")
                m3 = small.tile([P, 8], f32, tag="m3")
                nc.vector.max(out=m1, in_=cand)
                nc.vector.match_replace(out=cand, in_to_replace=m1,
                                        in_values=cand, imm_value=NEG_INF)
                nc.vector.max(out=m2, in_=cand)
                nc.vector.match_replace(out=cand, in_to_replace=m2,
                                        in_values=cand, imm_value=NEG_INF)
                nc.vector.max(out=m3, in_=cand)
                tau = m3[:, 3:4]    # 20th largest; m3[:, 4] = 21st
                nc.gpsimd.tensor_copy(out=OUT[:, s0 + 12:s0 + 14],
                                      in_=m3[:, 3:5])

                # masked sum: ME_c = (L_c >= tau) * e_c, accum per chunk
                for c in range(NCH):
                    cl = c * CS
                    MEc = pLT.tile([P, CS], f32, tag="LT")
                    nc.vector.scalar_tensor_tensor(
                        out=MEc, in0=Lt[:, cl:cl + CS], scalar=tau,
                        in1=EB[:, cl:cl + CS],
                        op0=OP.is_ge, op1=OP.mult,
                        accum_out=OUT[:, s0 + 8 + c:s0 + 9 + c])

            nc.sync.dma_start(out.ap(), OUT)

    # Force every activation onto one table (Exp+Ln+Copy live together in
    # natural_log_exp_and_others) so the engine never reloads tables.
    # get_activation_tables is functools.cached -> mutate the cached dict
    # during compile, then restore.
    tabs = get_activation_tables(nc.m.arch)
    saved = {k: set(v) for k, v in tabs.items()}
    try:
        for k in tabs:
            if k != ACT_TABLE:
                tabs[k] = set()
        nc.compile()
    finally:
        for k, v in saved.items():
            tabs[k] = v
    return nc


def _get_program():
    global _PROGRAM
    if _PROGRAM is None:
        _PROGRAM = _build_program()
    return _PROGRAM


def _run_on_cores(logits, targets, trace=False, **kw):
    from concourse import bass_utils
    nc = _get_program()
    in_maps = [
        {"logits": np.ascontiguousarray(logits[c * R:(c + 1) * R]),
         "targets": np.ascontiguousarray(targets[c * R:(c + 1) * R])}
        for c in range(NCORES)
    ]
    return bass_utils.run_bass_kernel_spmd(
        nc, in_maps, core_ids=list(range(NCORES)), trace=trace, **kw)


def _host_fix_rows(logits, targets, rows):
    """Exact per-row recompute of the top-20 term, replicating the
    reference's tie-breaking (top_k on f32 sigmoid, stable by index)."""
    out = {}
    for r in rows:
        l = logits[r].astype(np.float32)
        t = targets[r].astype(np.float64)
        p = (1.0 / (1.0 + np.exp(-l.astype(np.float64)))).astype(np.float32)
        idx = np.argsort(-p, kind="stable")[:K]
        ld = l[idx].astype(np.float64)
        td = t[idx]
        sp = np.maximum(ld, 0) + np.log1p(np.exp(-np.abs(ld)))
        out[r] = float(np.sum(sp - ld * td))
    return out


def kernel(logits, targets, BCE_L):
    logits = np.asarray(logits, dtype=np.float32)
    targets = np.asarray(targets, dtype=np.float32)
    res = _run_on_cores(logits, targets)
    # partials[core]: [P, NT*SLOTS]; global row = core*R + t*P + p
    bce_sum = 0.0
    me = np.zeros((NCORES, NT, P), dtype=np.float64)
    flag = np.zeros((NCORES, NT, P), dtype=bool)
    for c in range(NCORES):
        par = res.results[c]["partials"].astype(np.float64)
        for t in range(NT):
            s0 = t * SLOTS
            bce_sum += float(np.sum(par[:, s0:s0 + 4])
                             - np.sum(par[:, s0 + 4:s0 + 8]))
            me[c, t] = par[:, s0 + 8:s0 + 12].sum(axis=1)
            tau = par[:, s0 + 12]
            tau2 = par[:, s0 + 13]
            ch8max = par[:, s0 + 16:s0 + 32].max(axis=1)
            flag[c, t] = (ch8max >= tau) | (tau2 == tau)
    me_rows = me.reshape(-1)
    bad = np.nonzero(flag.reshape(-1))[0]
    if bad.size:
        fixes = _host_fix_rows(logits, targets, bad.tolist())
        for r, v in fixes.items():
            me_rows[r] = v
    out = bce_sum / (B * N) + float(BCE_L[0]) * float(me_rows.sum()) / (B * K)
    return np.array(out, dtype=np.float32)


# revision 9
# speedup vs baseline: 1.0341x; 1.0341x over previous
"""Trainium2 Bass kernel for CustomBCEWithLogitsLoss (topk masking).

Math: with e = softplus(l) - l*t (elementwise BCE-with-logits),
  out = mean_all(e) + BCE_L * mean_{top20-by-logit per row}(e)
since top-k of sigmoid(logits) = top-k of logits, and the reference's
top-k BCE term equals e at those positions (-100 clamps never bind for
|l| < 100). Decompose further:
  sum_all e  = sum softplus(l) - sum l*t
  sum_top e  = sum softplus(top values) - sum_top l*t
The top-20 VALUES come straight from the max8 cascade, so softplus needs
no masked pass - only sum_top(l*t) does.

Per core (8-way batch shard, 512 rows = 4 tiles of [128, 10000]):
  DMA(SP): L and T in half-row DMAs (2.56MB each)
  GPSIMD: LT_h = L_h * T_h (the only bulk GPSIMD work)
  ACT:    softplus accum: Exp(L_h) over dead T_h, Ln(x+1) in place
          (accum -> sum sp); Copy(LT_h) (accum -> sum l*t); softplus of
          the 20 top values (accum -> sum_top sp). One activation table
          (natural_log_exp_and_others) serves Exp+Ln+Copy - no reloads.
  DVE:    16x max8 over 625-col chunks -> 128 candidates/row;
          3x(max8+match_replace) cascade -> top-24 values, tau = 20th;
          MLT_h = (L_h >= tau) * LT_h with accum -> sum_top l*t
Exactness: per-chunk 8th-largest (ch8) and the 21st candidate (tau2) are
output; host flags rows where max(ch8) >= tau (candidate set may have
missed a top-20 value) or tau2 == tau (boundary tie) and recomputes them
exactly (expected ~1 row in 1e5). Host combines partials in f64.
"""

import numpy as np

B, N, K = 4096, 10000, 20
NCORES = 8
R = B // NCORES          # rows per core
P = 128                  # partitions
NT = R // P              # tiles per core
H = N // 2               # half-row width
CCH = 16                 # candidate chunks per row
W = N // CCH             # candidate chunk width (625)
SLOTS = 32               # per-tile output slots
NEG_INF = -1.0e30
ACT_TABLE = "natural_log_exp_and_others"

_PROGRAM = None


def _build_program():
    import concourse.bacc as bacc
    import concourse.tile as tile
    import concourse.mybir as mybir
    from concourse.hw_specs import get_activation_tables

    nc = bacc.Bacc("TRN2", target_bir_lowering=False, debug=False)
    f32 = mybir.dt.float32
    logits = nc.dram_tensor("logits", [R, N], f32, kind="ExternalInput")
    targets = nc.dram_tensor("targets", [R, N], f32, kind="ExternalInput")
    out = nc.dram_tensor("partials", [P, NT * SLOTS], f32,
                         kind="ExternalOutput")
    Lr = logits.ap().rearrange("(t p) n -> t p n", p=P)
    Tr = targets.ap().rearrange("(t p) n -> t p n", p=P)

    AF = mybir.ActivationFunctionType
    OP = mybir.AluOpType

    with tile.TileContext(nc) as tc:
        with (
            tc.tile_pool(name="pL", bufs=2) as pL,
            tc.tile_pool(name="pT", bufs=2) as pT,
            tc.tile_pool(name="pLT", bufs=2) as pLT,
            tc.tile_pool(name="small", bufs=2) as small,
            tc.tile_pool(name="outp", bufs=1) as outp,
        ):
            OUT = outp.tile([P, NT * SLOTS], f32)
            nc.gpsimd.memset(OUT, 0.0)
            for t in range(NT):
                s0 = t * SLOTS
                Lt = pL.tile([P, N], f32, tag="L")
                nc.sync.dma_start(Lt[:, :H], Lr[t][:, :H])
                nc.sync.dma_start(Lt[:, H:], Lr[t][:, H:])
                LTh = []
                for h in range(2):
                    hl = h * H
                    Lh = Lt[:, hl:hl + H]
                    Th = pT.tile([P, H], f32, tag="T")
                    nc.sync.dma_start(Th, Tr[t][:, hl:hl + H])
                    LTc = pLT.tile([P, H], f32, tag="LT")
                    nc.gpsimd.tensor_mul(LTc, Lh, Th)
                    LTh.append(LTc)
                    # softplus accum over dead Th: Exp, then Ln(x+1) in place
                    nc.scalar.activation(Th, Lh, AF.Exp)
                    nc.scalar.activation(Th, Th, AF.Ln, bias=1.0, scale=1.0,
                                         accum_out=OUT[:, s0 + h:s0 + h + 1])
                    # row sum of l*t via ACT copy-accum (out over dead Th)
                    nc.scalar.activation(Th, LTc, AF.Copy,
                                         accum_out=OUT[:, s0 + 2 + h:s0 + 3 + h])

                # top-20: per-chunk top-8, then cascade on cand
                cand = small.tile([P, CCH * 8], f32, tag="cand")
                for c in range(CCH):
                    nc.vector.max(out=cand[:, c * 8:(c + 1) * 8],
                                  in_=Lt[:, c * W:(c + 1) * W])
                # 8th-largest of each chunk -> exactness check channel
                cv = cand[:].rearrange("p (c k) -> p c k", k=8)
                nc.gpsimd.tensor_copy(out=OUT[:, s0 + 16:s0 + 32],
                                      in_=cv[:, :, 7:8])
                mall = small.tile([P, 24], f32, tag="mall")
                nc.vector.max(out=mall[:, 0:8], in_=cand)
                nc.vector.match_replace(out=cand, in_to_replace=mall[:, 0:8],
                                        in_values=cand, imm_value=NEG_INF)
                nc.vector.max(out=mall[:, 8:16], in_=cand)
                nc.vector.match_replace(out=cand, in_to_replace=mall[:, 8:16],
                                        in_values=cand, imm_value=NEG_INF)
                nc.vector.max(out=mall[:, 16:24], in_=cand)
                tau = mall[:, 19:20]   # 20th largest; mall[:, 20] = 21st
                nc.gpsimd.tensor_copy(out=OUT[:, s0 + 12:s0 + 14],
                                      in_=mall[:, 19:21])

                # sum_top softplus from the top-20 values themselves
                x20 = small.tile([P, 20], f32, tag="x20")
                nc.scalar.activation(x20, mall[:, :20], AF.Exp)
                nc.scalar.activation(x20, x20, AF.Ln, bias=1.0, scale=1.0,
                                     accum_out=OUT[:, s0 + 6:s0 + 7])

                # masked sum: MLT_h = (L_h >= tau) * LT_h, accum per half
                for h in range(2):
                    hl = h * H
                    MEh = pT.tile([P, H], f32, tag="T")
                    nc.vector.scalar_tensor_tensor(
                        out=MEh, in0=Lt[:, hl:hl + H], scalar=tau,
                        in1=LTh[h],
                        op0=OP.is_ge, op1=OP.mult,
                        accum_out=OUT[:, s0 + 4 + h:s0 + 5 + h])

            nc.sync.dma_start(out.ap(), OUT)

    # Force every activation onto one table (Exp+Ln+Copy live together in
    # natural_log_exp_and_others) so the engine never reloads tables.
    tabs = get_activation_tables(nc.m.arch)
    saved = {k: set(v) for k, v in tabs.items()}
    try:
        for k in tabs:
            if k != ACT_TABLE:
                tabs[k] = set()
        nc.compile()
    finally:
        for k, v in saved.items():
            tabs[k] = v
    return nc


def _get_program():
    global _PROGRAM
    if _PROGRAM is None:
        _PROGRAM = _build_program()
    return _PROGRAM


def _run_on_cores(logits, targets, trace=False, **kw):
    from concourse import bass_utils
    nc = _get_program()
    in_maps = [
        {"logits": np.ascontiguousarray(logits[c * R:(c + 1) * R]),
         "targets": np.ascontiguousarray(targets[c * R:(c + 1) * R])}
        for c in range(NCORES)
    ]
    return bass_utils.run_bass_kernel_spmd(
        nc, in_maps, core_ids=list(range(NCORES)), trace=trace, **kw)


def _host_fix_rows(logits, targets, rows):
    """Exact per-row recompute of the top-20 term, replicating the
    reference's tie-breaking (top_k on f32 sigmoid, stable by index)."""
    out = {}
    for r in rows:
        l = logits[r].astype(np.float32)
        t = targets[r].astype(np.float64)
        p = (1.0 / (1.0 + np.exp(-l.astype(np.float64)))).astype(np.float32)
        idx = np.argsort(-p, kind="stable")[:K]
        ld = l[idx].astype(np.float64)
        td = t[idx]
        sp = np.maximum(ld, 0) + np.log1p(np.exp(-np.abs(ld)))
        out[r] = float(np.sum(sp - ld * td))
    return out


def kernel(logits, targets, BCE_L):
    logits = np.asarray(logits, dtype=np.float32)
    targets = np.asarray(targets, dtype=np.float32)
    res = _run_on_cores(logits, targets)
    # partials[core]: [P, NT*SLOTS]; global row = core*R + t*P + p
    # slots: 0-1 sum sp halves, 2-3 sum lt halves, 4-5 masked lt halves,
    #        6 sum_top sp, 12 tau, 13 tau2, 16-31 ch8
    bce_sum = 0.0
    me = np.zeros((NCORES, NT, P), dtype=np.float64)
    flag = np.zeros((NCORES, NT, P), dtype=bool)
    for c in range(NCORES):
        par = res.results[c]["partials"].astype(np.float64)
        for t in range(NT):
            s0 = t * SLOTS
            bce_sum += float(np.sum(par[:, s0:s0 + 2])
                             - np.sum(par[:, s0 + 2:s0 + 4]))
            me[c, t] = par[:, s0 + 6] - par[:, s0 + 4] - par[:, s0 + 5]
            tau = par[:, s0 + 12]
            tau2 = par[:, s0 + 13]
            ch8max = par[:, s0 + 16:s0 + 32].max(axis=1)
            flag[c, t] = (ch8max >= tau) | (tau2 == tau)
    me_rows = me.reshape(-1)
    bad = np.nonzero(flag.reshape(-1))[0]
    if bad.size:
        fixes = _host_fix_rows(logits, targets, bad.tolist())
        for r, v in fixes.items():
            me_rows[r] = v
    out = bce_sum / (B * N) + float(BCE_L[0]) * float(me_rows.sum()) / (B * K)
    return np.array(out, dtype=np.float32)


# revision 14
# speedup vs baseline: 1.1684x; 1.1299x over previous
"""Trainium2 Bass kernel for CustomBCEWithLogitsLoss (topk masking).

Math: with e = softplus(l) - l*t (elementwise BCE-with-logits),
  out = mean_all(e) + BCE_L * mean_{top20-by-logit per row}(e)
since top-k of sigmoid(logits) = top-k of logits, and the reference's
top-k BCE term equals e at those positions (-100 clamps never bind for
|l| < 100). Decompose further:
  sum_all e  = sum softplus(l) - sum l*t
  sum_top e  = sum softplus(top values) - sum_top l*t
The top-20 VALUES come straight from the max8 cascade, so softplus needs
no masked pass - only sum_top(l*t) does.

Per core (8-way batch shard, 512 rows = 4 tiles of [128, 10000]):
  DMA(SP): L and T in half-row DMAs (2.56MB each)
  GPSIMD: LT_h = L_h * T_h (the only bulk GPSIMD work)
  ACT:    softplus accum: Exp(L_h) over dead T_h, Ln(x+1) in place
          (accum -> sum sp); Copy(LT_h) (accum -> sum l*t); softplus of
          the 20 top values (accum -> sum_top sp). One activation table
          (natural_log_exp_and_others) serves Exp+Ln+Copy - no reloads.
  DVE:    16x max8 over 625-col chunks -> 128 candidates/row;
          3x(max8+match_replace) cascade -> top-24 values, tau = 20th;
          MLT_h = (L_h >= tau) * LT_h with accum -> sum_top l*t
Exactness: per-chunk 8th-largest (ch8) and the 21st candidate (tau2) are
output; host flags rows where max(ch8) >= tau (candidate set may have
missed a top-20 value) or tau2 == tau (boundary tie) and recomputes them
exactly (expected ~1 row in 1e5). Host combines partials in f64.
"""

import numpy as np

B, N, K = 4096, 10000, 20
NCORES = 8
R = B // NCORES          # rows per core
P = 128                  # partitions
NT = R // P              # tiles per core
H = N // 2               # half-row width
CCH = 16                 # candidate chunks per row
W = N // CCH             # candidate chunk width (625)
SLOTS = 32               # per-tile output slots
NEG_INF = -1.0e30
ACT_TABLE = "natural_log_exp_and_others"

_PROGRAM = None


def _build_program():
    import concourse.bacc as bacc
    import concourse.tile as tile
    import concourse.mybir as mybir
    from concourse.hw_specs import get_activation_tables

    nc = bacc.Bacc("TRN2", target_bir_lowering=False, debug=False)
    f32 = mybir.dt.float32
    logits = nc.dram_tensor("logits", [R, N], f32, kind="ExternalInput")
    targets = nc.dram_tensor("targets", [R, N], f32, kind="ExternalInput")
    out = nc.dram_tensor("partials", [P, NT * SLOTS], f32,
                         kind="ExternalOutput")
    Lr = logits.ap().rearrange("(t p) n -> t p n", p=P)
    Tr = targets.ap().rearrange("(t p) n -> t p n", p=P)

    AF = mybir.ActivationFunctionType
    OP = mybir.AluOpType

    bf16 = mybir.dt.bfloat16
    with tile.TileContext(nc) as tc:
        with (
            tc.tile_pool(name="pL", bufs=2) as pL,
            tc.tile_pool(name="pT", bufs=2) as pT,
            tc.tile_pool(name="pLT", bufs=2) as pLT,
            tc.tile_pool(name="pSP", bufs=1) as pSP,
            tc.tile_pool(name="cnd", bufs=1) as cnd,
            tc.tile_pool(name="small", bufs=2) as small,
            tc.tile_pool(name="outp", bufs=1) as outp,
        ):
            OUT = outp.tile([P, NT * SLOTS], f32)
            nc.gpsimd.memset(OUT, 0.0)
            for t in range(NT):
                s0 = t * SLOTS
                Lt = pL.tile([P, N], f32, tag="L")
                LTh = []
                for h in range(2):
                    hl = h * H
                    Lh = Lt[:, hl:hl + H]
                    nc.sync.dma_start(Lh, Lr[t][:, hl:hl + H])
                    Th = pT.tile([P, H], f32, tag="T")
                    nc.sync.dma_start(Th, Tr[t][:, hl:hl + H])
                    LTc = pLT.tile([P, H], f32, tag="LT")
                    nc.gpsimd.tensor_mul(LTc, Lh, Th)
                    LTh.append(LTc)
                    # softplus accum via bf16 ACT scratch (sum stays f32)
                    SPh = pSP.tile([P, H], bf16, tag="SP")
                    nc.scalar.activation(SPh, Lh, AF.Exp)
                    nc.scalar.activation(SPh, SPh, AF.Ln, bias=1.0, scale=1.0,
                                         accum_out=OUT[:, s0 + h:s0 + h + 1])
                    # row sum of l*t via ACT copy-accum (out over scratch)
                    nc.scalar.activation(SPh, LTc, AF.Copy,
                                         accum_out=OUT[:, s0 + 2 + h:s0 + 3 + h])

                # top-20: per-chunk top-8, then cascade on cand
                cand = cnd.tile([P, CCH * 8], f32, tag="cand")
                for c in range(CCH):
                    nc.vector.max(out=cand[:, c * 8:(c + 1) * 8],
                                  in_=Lt[:, c * W:(c + 1) * W])
                # 8th-largest of each chunk -> exactness check channel
                cv = cand[:].rearrange("p (c k) -> p c k", k=8)
                nc.gpsimd.tensor_copy(out=OUT[:, s0 + 16:s0 + 32],
                                      in_=cv[:, :, 7:8])
                mall = small.tile([P, 48], f32, tag="mall")
                nc.vector.max(out=mall[:, 0:8], in_=cand)
                nc.vector.match_replace(out=cand, in_to_replace=mall[:, 0:8],
                                        in_values=cand, imm_value=NEG_INF)
                nc.vector.max(out=mall[:, 8:16], in_=cand)
                nc.vector.match_replace(out=cand, in_to_replace=mall[:, 8:16],
                                        in_values=cand, imm_value=NEG_INF)
                nc.vector.max(out=mall[:, 16:24], in_=cand)
                tau = mall[:, 19:20]   # 20th largest; mall[:, 20] = 21st
                nc.gpsimd.tensor_copy(out=OUT[:, s0 + 12:s0 + 14],
                                      in_=mall[:, 19:21])

                # sum_top softplus from the top-20 values themselves
                x20 = mall[:, 24:44]
                nc.scalar.activation(x20, mall[:, :20], AF.Exp)
                nc.scalar.activation(x20, x20, AF.Ln, bias=1.0, scale=1.0,
                                     accum_out=OUT[:, s0 + 6:s0 + 7])

                # masked sum: MLT_h = (L_h >= tau) * LT_h, accum per half,
                # written in place over the LT input (releases the slot)
                for h in range(2):
                    hl = h * H
                    nc.vector.scalar_tensor_tensor(
                        out=LTh[h], in0=Lt[:, hl:hl + H], scalar=tau,
                        in1=LTh[h],
                        op0=OP.is_ge, op1=OP.mult,
                        accum_out=OUT[:, s0 + 4 + h:s0 + 5 + h])

            nc.sync.dma_start(out.ap(), OUT)

    # Force every activation onto one table (Exp+Ln+Copy live together in
    # natural_log_exp_and_others) so the engine never reloads tables.
    tabs = get_activation_tables(nc.m.arch)
    saved = {k: set(v) for k, v in tabs.items()}
    try:
        for k in tabs:
            if k != ACT_TABLE:
                tabs[k] = set()
        nc.compile()
    finally:
        for k, v in saved.items():
            tabs[k] = v
    return nc


def _get_program():
    global _PROGRAM
    if _PROGRAM is None:
        _PROGRAM = _build_program()
    return _PROGRAM


def _run_on_cores(logits, targets, trace=False, **kw):
    from concourse import bass_utils
    nc = _get_program()
    in_maps = [
        {"logits": np.ascontiguousarray(logits[c * R:(c + 1) * R]),
         "targets": np.ascontiguousarray(targets[c * R:(c + 1) * R])}
        for c in range(NCORES)
    ]
    return bass_utils.run_bass_kernel_spmd(
        nc, in_maps, core_ids=list(range(NCORES)), trace=trace, **kw)


def _host_fix_rows(logits, targets, rows):
    """Exact per-row recompute of the top-20 term, replicating the
    reference's tie-breaking (top_k on f32 sigmoid, stable by index)."""
    out = {}
    for r in rows:
        l = logits[r].astype(np.float32)
        t = targets[r].astype(np.float64)
        p = (1.0 / (1.0 + np.exp(-l.astype(np.float64)))).astype(np.float32)
        idx = np.argsort(-p, kind="stable")[:K]
        ld = l[idx].astype(np.float64)
        td = t[idx]
        sp = np.maximum(ld, 0) + np.log1p(np.exp(-np.abs(ld)))
        out[r] = float(np.sum(sp - ld * td))
    return out


def kernel(logits, targets, BCE_L):
    logits = np.asarray(logits, dtype=np.float32)
    targets = np.asarray(targets, dtype=np.float32)
    res = _run_on_cores(logits, targets)
    # partials[core]: [P, NT*SLOTS]; global row = core*R + t*P + p
    # slots: 0-1 sum sp halves, 2-3 sum lt halves, 4-5 masked lt halves,
    #        6 sum_top sp, 12 tau, 13 tau2, 16-31 ch8
    bce_sum = 0.0
    me = np.zeros((NCORES, NT, P), dtype=np.float64)
    flag = np.zeros((NCORES, NT, P), dtype=bool)
    for c in range(NCORES):
        par = res.results[c]["partials"].astype(np.float64)
        for t in range(NT):
            s0 = t * SLOTS
            bce_sum += float(np.sum(par[:, s0:s0 + 2])
                             - np.sum(par[:, s0 + 2:s0 + 4]))
            me[c, t] = par[:, s0 + 6] - par[:, s0 + 4] - par[:, s0 + 5]
            tau = par[:, s0 + 12]
            tau2 = par[:, s0 + 13]
            ch8max = par[:, s0 + 16:s0 + 32].max(axis=1)
            flag[c, t] = (ch8max >= tau) | (tau2 == tau)
    me_rows = me.reshape(-1)
    bad = np.nonzero(flag.reshape(-1))[0]
    if bad.size:
        fixes = _host_fix_rows(logits, targets, bad.tolist())
        for r, v in fixes.items():
            me_rows[r] = v
    out = bce_sum / (B * N) + float(BCE_L[0]) * float(me_rows.sum()) / (B * K)
    return np.array(out, dtype=np.float32)


# revision 15
# speedup vs baseline: 1.1928x; 1.0209x over previous
"""Trainium2 Bass kernel for CustomBCEWithLogitsLoss (topk masking).

Math: with e = softplus(l) - l*t (elementwise BCE-with-logits),
  out = mean_all(e) + BCE_L * mean_{top20-by-logit per row}(e)
since top-k of sigmoid(logits) = top-k of logits, and the reference's
top-k BCE term equals e at those positions (-100 clamps never bind for
|l| < 100). Decompose further:
  sum_all e  = sum softplus(l) - sum l*t
  sum_top e  = sum softplus(top values) - sum_top l*t
The top-20 VALUES come straight from the max8 cascade, so softplus needs
no masked pass - only sum_top(l*t) does.

Per core (8-way batch shard, 512 rows = 4 tiles of [128, 10000]):
  DMA(SP): L and T in half-row DMAs (2.56MB each)
  GPSIMD: LT_h = L_h * T_h (the only bulk GPSIMD work)
  ACT:    softplus accum: Exp(L_h) over dead T_h, Ln(x+1) in place
          (accum -> sum sp); Copy(LT_h) (accum -> sum l*t); softplus of
          the 20 top values (accum -> sum_top sp). One activation table
          (natural_log_exp_and_others) serves Exp+Ln+Copy - no reloads.
  DVE:    16x max8 over 625-col chunks -> 128 candidates/row;
          3x(max8+match_replace) cascade -> top-24 values, tau = 20th;
          MLT_h = (L_h >= tau) * LT_h with accum -> sum_top l*t
Exactness: per-chunk 8th-largest (ch8) and the 21st candidate (tau2) are
output; host flags rows where max(ch8) >= tau (candidate set may have
missed a top-20 value) or tau2 == tau (boundary tie) and recomputes them
exactly (expected ~1 row in 1e5). Host combines partials in f64.
"""

import numpy as np

B, N, K = 4096, 10000, 20
NCORES = 8
R = B // NCORES          # rows per core
P = 128                  # partitions
NT = R // P              # tiles per core
H = N // 2               # half-row width
CCH = 16                 # candidate chunks per row
W = N // CCH             # candidate chunk width (625)
SLOTS = 32               # per-tile output slots
NEG_INF = -1.0e30
ACT_TABLE = "natural_log_exp_and_others"

_PROGRAM = None


def _build_program():
    import concourse.bacc as bacc
    import concourse.tile as tile
    import concourse.mybir as mybir
    from concourse.hw_specs import get_activation_tables

    nc = bacc.Bacc("TRN2", target_bir_lowering=False, debug=False)
    f32 = mybir.dt.float32
    logits = nc.dram_tensor("logits", [R, N], f32, kind="ExternalInput")
    targets = nc.dram_tensor("targets", [R, N], f32, kind="ExternalInput")
    out = nc.dram_tensor("partials", [P, NT * SLOTS], f32,
                         kind="ExternalOutput")
    Lr = logits.ap().rearrange("(t p) n -> t p n", p=P)
    Tr = targets.ap().rearrange("(t p) n -> t p n", p=P)

    AF = mybir.ActivationFunctionType
    OP = mybir.AluOpType

    bf16 = mybir.dt.bfloat16
    with tile.TileContext(nc) as tc:
        with (
            tc.tile_pool(name="pL", bufs=2) as pL,
            tc.tile_pool(name="pT", bufs=2) as pT,
            tc.tile_pool(name="pLT", bufs=4) as pLT,
            tc.tile_pool(name="pSP", bufs=1) as pSP,
            tc.tile_pool(name="cnd", bufs=1) as cnd,
            tc.tile_pool(name="small", bufs=2) as small,
            tc.tile_pool(name="outp", bufs=1) as outp,
        ):
            OUT = outp.tile([P, NT * SLOTS], f32)
            nc.gpsimd.memset(OUT, 0.0)
            for t in range(NT):
                s0 = t * SLOTS
                Lt = pL.tile([P, N], f32, tag="L")
                LTh = []
                for h in range(2):
                    hl = h * H
                    Lh = Lt[:, hl:hl + H]
                    nc.sync.dma_start(Lh, Lr[t][:, hl:hl + H])
                    Th = pT.tile([P, H], f32, tag="T")
                    nc.sync.dma_start(Th, Tr[t][:, hl:hl + H])
                    LTc = pLT.tile([P, H], bf16, tag="LT")
                    nc.gpsimd.tensor_mul(LTc, Lh, Th)
                    LTh.append(LTc)
                    # softplus accum via bf16 ACT scratch (sum stays f32)
                    SPh = pSP.tile([P, H], bf16, tag="SP")
                    nc.scalar.activation(SPh, Lh, AF.Exp)
                    nc.scalar.activation(SPh, SPh, AF.Ln, bias=1.0, scale=1.0,
                                         accum_out=OUT[:, s0 + h:s0 + h + 1])
                    # row sum of l*t via ACT copy-accum (out over scratch)
                    nc.scalar.activation(SPh, LTc, AF.Copy,
                                         accum_out=OUT[:, s0 + 2 + h:s0 + 3 + h])

                # top-20: per-chunk top-8, then cascade on cand
                cand = cnd.tile([P, CCH * 8], f32, tag="cand")
                for c in range(CCH):
                    nc.vector.max(out=cand[:, c * 8:(c + 1) * 8],
                                  in_=Lt[:, c * W:(c + 1) * W])
                # 8th-largest of each chunk -> exactness check channel
                cv = cand[:].rearrange("p (c k) -> p c k", k=8)
                nc.gpsimd.tensor_copy(out=OUT[:, s0 + 16:s0 + 32],
                                      in_=cv[:, :, 7:8])
                mall = small.tile([P, 48], f32, tag="mall")
                nc.vector.max(out=mall[:, 0:8], in_=cand)
                nc.vector.match_replace(out=cand, in_to_replace=mall[:, 0:8],
                                        in_values=cand, imm_value=NEG_INF)
                nc.vector.max(out=mall[:, 8:16], in_=cand)
                nc.vector.match_replace(out=cand, in_to_replace=mall[:, 8:16],
                                        in_values=cand, imm_value=NEG_INF)
                nc.vector.max(out=mall[:, 16:24], in_=cand)
                tau = mall[:, 19:20]   # 20th largest; mall[:, 20] = 21st
                nc.gpsimd.tensor_copy(out=OUT[:, s0 + 12:s0 + 14],
                                      in_=mall[:, 19:21])

                # sum_top softplus from the top-20 values themselves
                x20 = mall[:, 24:44]
                nc.scalar.activation(x20, mall[:, :20], AF.Exp)
                nc.scalar.activation(x20, x20, AF.Ln, bias=1.0, scale=1.0,
                                     accum_out=OUT[:, s0 + 6:s0 + 7])

                # masked sum: MLT_h = (L_h >= tau) * LT_h, accum per half,
                # written in place over the LT input (releases the slot)
                for h in range(2):
                    hl = h * H
                    nc.vector.scalar_tensor_tensor(
                        out=LTh[h], in0=Lt[:, hl:hl + H], scalar=tau,
                        in1=LTh[h],
                        op0=OP.is_ge, op1=OP.mult,
                        accum_out=OUT[:, s0 + 4 + h:s0 + 5 + h])

            nc.sync.dma_start(out.ap(), OUT)

    # Force every activation onto one table (Exp+Ln+Copy live together in
    # natural_log_exp_and_others) so the engine never reloads tables.
    tabs = get_activation_tables(nc.m.arch)
    saved = {k: set(v) for k, v in tabs.items()}
    try:
        for k in tabs:
            if k != ACT_TABLE:
                tabs[k] = set()
        nc.compile()
    finally:
        for k, v in saved.items():
            tabs[k] = v
    return nc


def _get_program():
    global _PROGRAM
    if _PROGRAM is None:
        _PROGRAM = _build_program()
    return _PROGRAM


def _run_on_cores(logits, targets, trace=False, **kw):
    from concourse import bass_utils
    nc = _get_program()
    in_maps = [
        {"logits": np.ascontiguousarray(logits[c * R:(c + 1) * R]),
         "targets": np.ascontiguousarray(targets[c * R:(c + 1) * R])}
        for c in range(NCORES)
    ]
    return bass_utils.run_bass_kernel_spmd(
        nc, in_maps, core_ids=list(range(NCORES)), trace=trace, **kw)


def _host_fix_rows(logits, targets, rows):
    """Exact per-row recompute of the top-20 term, replicating the
    reference's tie-breaking (top_k on f32 sigmoid, stable by index)."""
    out = {}
    for r in rows:
        l = logits[r].astype(np.float32)
        t = targets[r].astype(np.float64)
        p = (1.0 / (1.0 + np.exp(-l.astype(np.float64)))).astype(np.float32)
        idx = np.argsort(-p, kind="stable")[:K]
        ld = l[idx].astype(np.float64)
        td = t[idx]
        sp = np.maximum(ld, 0) + np.log1p(np.exp(-np.abs(ld)))
        out[r] = float(np.sum(sp - ld * td))
    return out


def kernel(logits, targets, BCE_L):
    logits = np.asarray(logits, dtype=np.float32)
    targets = np.asarray(targets, dtype=np.float32)
    res = _run_on_cores(logits, targets)
    # partials[core]: [P, NT*SLOTS]; global row = core*R + t*P + p
    # slots: 0-1 sum sp halves, 2-3 sum lt halves, 4-5 masked lt halves,
    #        6 sum_top sp, 12 tau, 13 tau2, 16-31 ch8
    bce_sum = 0.0
    me = np.zeros((NCORES, NT, P), dtype=np.float64)
    flag = np.zeros((NCORES, NT, P), dtype=bool)
    for c in range(NCORES):
        par = res.results[c]["partials"].astype(np.float64)
        for t in range(NT):
            s0 = t * SLOTS
            bce_sum += float(np.sum(par[:, s0:s0 + 2])
                             - np.sum(par[:, s0 + 2:s0 + 4]))
            me[c, t] = par[:, s0 + 6] - par[:, s0 + 4] - par[:, s0 + 5]
            tau = par[:, s0 + 12]
            tau2 = par[:, s0 + 13]
            ch8max = par[:, s0 + 16:s0 + 32].max(axis=1)
            flag[c, t] = (ch8max >= tau) | (tau2 == tau)
    me_rows = me.reshape(-1)
    bad = np.nonzero(flag.reshape(-1))[0]
    if bad.size:
        fixes = _host_fix_rows(logits, targets, bad.tolist())
        for r, v in fixes.items():
            me_rows[r] = v
    out = bce_sum / (B * N) + float(BCE_L[0]) * float(me_rows.sum()) / (B * K)
    return np.array(out, dtype=np.float32)


# revision 18
# speedup vs baseline: 1.4257x; 1.1953x over previous
"""Trainium2 Bass kernel for CustomBCEWithLogitsLoss (topk masking).

Math: with e = softplus(l) - l*t (elementwise BCE-with-logits),
  out = mean_all(e) + BCE_L * mean_{top20-by-logit per row}(e)
since top-k of sigmoid(logits) = top-k of logits, and the reference's
top-k BCE term equals e at those positions (-100 clamps never bind for
|l| < 100). Decompose further:
  sum_all e  = sum softplus(l) - sum l*t
  sum_top e  = sum softplus(top values) - sum_top l*t
The top-20 VALUES come straight from the max8 cascade, so softplus needs
no masked pass - only sum_top(l*t) does.

Per core (8-way batch shard, 512 rows = 4 tiles of [128, 10000]):
  DMA(SP): L and T in half-row DMAs (2.56MB each)
  GPSIMD: LT_h = L_h * T_h (the only bulk GPSIMD work)
  ACT:    softplus accum: Exp(L_h) over dead T_h, Ln(x+1) in place
          (accum -> sum sp); Copy(LT_h) (accum -> sum l*t); softplus of
          the 20 top values (accum -> sum_top sp). One activation table
          (natural_log_exp_and_others) serves Exp+Ln+Copy - no reloads.
  DVE:    16x max8 over 625-col chunks -> 128 candidates/row;
          3x(max8+match_replace) cascade -> top-24 values, tau = 20th;
          MLT_h = (L_h >= tau) * LT_h with accum -> sum_top l*t
Exactness: per-chunk 8th-largest (ch8) and the 21st candidate (tau2) are
output; host flags rows where max(ch8) >= tau (candidate set may have
missed a top-20 value) or tau2 == tau (boundary tie) and recomputes them
exactly (expected ~1 row in 1e5). Host combines partials in f64.
"""

import numpy as np

B, N, K = 4096, 10000, 20
NCORES = 8
R = B // NCORES          # rows per core
P = 128                  # partitions
NT = R // P              # tiles per core
H = N // 2               # half-row width
CCH = 16                 # candidate chunks per row
W = N // CCH             # candidate chunk width (625)
SLOTS = 32               # per-tile output slots
NEG_INF = -1.0e30
ACT_TABLE = "natural_log_exp_and_others"

_PROGRAM = None


def _build_program():
    import concourse.bacc as bacc
    import concourse.tile as tile
    import concourse.mybir as mybir
    from concourse.hw_specs import get_activation_tables

    nc = bacc.Bacc("TRN2", target_bir_lowering=False, debug=False)
    f32 = mybir.dt.float32
    logits = nc.dram_tensor("logits", [R, N], f32, kind="ExternalInput")
    targets = nc.dram_tensor("targets", [R, N], f32, kind="ExternalInput")
    out = nc.dram_tensor("partials", [P, NT * SLOTS], f32,
                         kind="ExternalOutput")
    Lr = logits.ap().rearrange("(t p) n -> t p n", p=P)
    Tr = targets.ap().rearrange("(t p) n -> t p n", p=P)

    AF = mybir.ActivationFunctionType
    OP = mybir.AluOpType

    bf16 = mybir.dt.bfloat16
    with tile.TileContext(nc) as tc:
        with (
            tc.tile_pool(name="pL", bufs=2) as pL,
            tc.tile_pool(name="pT", bufs=2) as pT,
            tc.tile_pool(name="pLT", bufs=4) as pLT,
            tc.tile_pool(name="pSP", bufs=1) as pSP,
            tc.tile_pool(name="cnd", bufs=1) as cnd,
            tc.tile_pool(name="small", bufs=2) as small,
            tc.tile_pool(name="outp", bufs=1) as outp,
        ):
            OUT = outp.tile([P, NT * SLOTS], f32)
            nc.gpsimd.memset(OUT, 0.0)
            for t in range(NT):
                s0 = t * SLOTS
                Lt = pL.tile([P, N], f32, tag="L")
                LTh = []
                for h in range(2):
                    hl = h * H
                    Lh = Lt[:, hl:hl + H]
                    nc.sync.dma_start(Lh, Lr[t][:, hl:hl + H])
                    Th = pT.tile([P, H], f32, tag="T")
                    nc.sync.dma_start(Th, Tr[t][:, hl:hl + H])
                    LTc = pLT.tile([P, H], bf16, tag="LT")
                    if h == 0:
                        # GPSIMD computes this half; ACT reduces it
                        nc.gpsimd.tensor_mul(LTc, Lh, Th)
                    else:
                        # DVE computes this half with fused row-sum accum
                        for q in range(2):
                            ql = q * (H // 2)
                            nc.vector.scalar_tensor_tensor(
                                out=LTc[:, ql:ql + H // 2],
                                in0=Lh[:, ql:ql + H // 2], scalar=1.0,
                                in1=Th[:, ql:ql + H // 2],
                                op0=OP.mult, op1=OP.mult,
                                accum_out=OUT[:, s0 + 14 + q:s0 + 15 + q])
                    LTh.append(LTc)
                    # softplus accum via bf16 ACT scratch (sum stays f32)
                    SPh = pSP.tile([P, H], bf16, tag="SP")
                    nc.scalar.activation(SPh, Lh, AF.Exp)
                    nc.scalar.activation(SPh, SPh, AF.Ln, bias=1.0, scale=1.0,
                                         accum_out=OUT[:, s0 + h:s0 + h + 1])
                    if h == 0:
                        # row sum of l*t (GPSIMD half) via ACT copy-accum
                        nc.scalar.activation(SPh, LTc, AF.Copy,
                                             accum_out=OUT[:, s0 + 2:s0 + 3])

                # top-20: per-chunk top-8, then cascade on cand
                cand = cnd.tile([P, CCH * 8], f32, tag="cand")
                for c in range(CCH):
                    nc.vector.max(out=cand[:, c * 8:(c + 1) * 8],
                                  in_=Lt[:, c * W:(c + 1) * W])
                # 8th-largest of each chunk -> exactness check channel
                cv = cand[:].rearrange("p (c k) -> p c k", k=8)
                nc.gpsimd.tensor_copy(out=OUT[:, s0 + 16:s0 + 32],
                                      in_=cv[:, :, 7:8])
                mall = small.tile([P, 48], f32, tag="mall")
                nc.vector.max(out=mall[:, 0:8], in_=cand)
                nc.vector.match_replace(out=cand, in_to_replace=mall[:, 0:8],
                                        in_values=cand, imm_value=NEG_INF)
                nc.vector.max(out=mall[:, 8:16], in_=cand)
                nc.vector.match_replace(out=cand, in_to_replace=mall[:, 8:16],
                                        in_values=cand, imm_value=NEG_INF)
                nc.vector.max(out=mall[:, 16:24], in_=cand)
                tau = mall[:, 19:20]   # 20th largest; mall[:, 20] = 21st
                nc.gpsimd.tensor_copy(out=OUT[:, s0 + 12:s0 + 14],
                                      in_=mall[:, 19:21])

                # sum_top softplus from the top-20 values themselves
                x20 = mall[:, 24:44]
                nc.scalar.activation(x20, mall[:, :20], AF.Exp)
                nc.scalar.activation(x20, x20, AF.Ln, bias=1.0, scale=1.0,
                                     accum_out=OUT[:, s0 + 6:s0 + 7])

                # masked sum: MLT = (L >= tau) * LT in quarter chunks, accum
                # each, written in place over the LT input (releases slots)
                for c in range(4):
                    h, q = c // 2, c % 2
                    hl = h * H + q * (H // 2)
                    sl = slice(q * (H // 2), (q + 1) * (H // 2))
                    nc.vector.scalar_tensor_tensor(
                        out=LTh[h][:, sl], in0=Lt[:, hl:hl + H // 2],
                        scalar=tau, in1=LTh[h][:, sl],
                        op0=OP.is_ge, op1=OP.mult,
                        accum_out=OUT[:, s0 + 8 + c:s0 + 9 + c])

            nc.sync.dma_start(out.ap(), OUT)

    # Force every activation onto one table (Exp+Ln+Copy live together in
    # natural_log_exp_and_others) so the engine never reloads tables.
    tabs = get_activation_tables(nc.m.arch)
    saved = {k: set(v) for k, v in tabs.items()}
    try:
        for k in tabs:
            if k != ACT_TABLE:
                tabs[k] = set()
        nc.compile()
    finally:
        for k, v in saved.items():
            tabs[k] = v
    return nc


def _get_program():
    global _PROGRAM
    if _PROGRAM is None:
        _PROGRAM = _build_program()
    return _PROGRAM


def _run_on_cores(logits, targets, trace=False, **kw):
    from concourse import bass_utils
    nc = _get_program()
    in_maps = [
        {"logits": np.ascontiguousarray(logits[c * R:(c + 1) * R]),
         "targets": np.ascontiguousarray(targets[c * R:(c + 1) * R])}
        for c in range(NCORES)
    ]
    return bass_utils.run_bass_kernel_spmd(
        nc, in_maps, core_ids=list(range(NCORES)), trace=trace, **kw)


def _host_fix_rows(logits, targets, rows):
    """Exact per-row recompute of the top-20 term, replicating the
    reference's tie-breaking (top_k on f32 sigmoid, stable by index)."""
    out = {}
    for r in rows:
        l = logits[r].astype(np.float32)
        t = targets[r].astype(np.float64)
        p = (1.0 / (1.0 + np.exp(-l.astype(np.float64)))).astype(np.float32)
        idx = np.argsort(-p, kind="stable")[:K]
        ld = l[idx].astype(np.float64)
        td = t[idx]
        sp = np.maximum(ld, 0) + np.log1p(np.exp(-np.abs(ld)))
        out[r] = float(np.sum(sp - ld * td))
    return out


def kernel(logits, targets, BCE_L):
    logits = np.asarray(logits, dtype=np.float32)
    targets = np.asarray(targets, dtype=np.float32)
    res = _run_on_cores(logits, targets)
    # partials[core]: [P, NT*SLOTS]; global row = core*R + t*P + p
    # slots: 0-1 sum sp halves, 2-3 sum lt halves, 4-5 masked lt halves,
    #        6 sum_top sp, 12 tau, 13 tau2, 16-31 ch8
    bce_sum = 0.0
    me = np.zeros((NCORES, NT, P), dtype=np.float64)
    flag = np.zeros((NCORES, NT, P), dtype=bool)
    for c in range(NCORES):
        par = res.results[c]["partials"].astype(np.float64)
        for t in range(NT):
            s0 = t * SLOTS
            bce_sum += float(np.sum(par[:, s0:s0 + 2])
                             - np.sum(par[:, s0 + 2:s0 + 3])
                             - np.sum(par[:, s0 + 14:s0 + 16]))
            me[c, t] = par[:, s0 + 6] - par[:, s0 + 8:s0 + 12].sum(axis=1)
            tau = par[:, s0 + 12]
            tau2 = par[:, s0 + 13]
            ch8max = par[:, s0 + 16:s0 + 32].max(axis=1)
            flag[c, t] = (ch8max >= tau) | (tau2 == tau)
    me_rows = me.reshape(-1)
    bad = np.nonzero(flag.reshape(-1))[0]
    if bad.size:
        fixes = _host_fix_rows(logits, targets, bad.tolist())
        for r, v in fixes.items():
            me_rows[r] = v
    out = bce_sum / (B * N) + float(BCE_L[0]) * float(me_rows.sum()) / (B * K)
    return np.array(out, dtype=np.float32)
